# revision 4
# baseline (speedup 1.0000x reference)
"""Bidirectional RNN (embed -> fwd/bwd tanh scans -> vocab projection) on 8
TRN2 NeuronCores.

Strategy (per core, SPMD, no collectives):
  - Every core runs BOTH recurrent scans redundantly for the full batch (the
    scan is serial in T and cheap relative to the fc projection; redundancy
    avoids any inter-core exchange of hidden states).
  - The fc projection + embedding table rows are consumed identically on all
    cores, but the big fc weight is column-split over vocab: core c receives
    W_fc[:, c*4000:(c+1)*4000] through its in_map and writes the matching
    output slab. Rank never appears in the program.
  - Host-side numpy does only data movement: token reordering, packing
    transposes, vocab slicing, and the (zero) b_fc broadcast add.

Layouts:
  - "packed" hidden state column block per step: col = m*16 + b for hidden
    row m*128+p  (m: 4 H-tiles, b: batch 16) -> one PSUM bank [128, 64].
  - hbig (per dir) [128, 4*513*16]: column m*8208 + slot*16 + b. Slots are
    original time indices; fwd fills slot t+1 left-to-right, bwd fills slot t
    right-to-left. fc reads 128-column contiguous runs as matmul weights.
"""
import numpy as np

import concourse.bacc as bacc
import concourse.bass as bass
import concourse.mybir as mybir
import concourse.tile as tile
from concourse.bass_utils import run_bass_kernel_spmd
from concourse.masks import make_identity

P = 128
VOCAB, EMBED, HIDDEN = 32000, 256, 512
B, T = 16, 512
NCORES = 8
VSLICE = VOCAB // NCORES          # 4000 vocab cols per core
NPANEL = 8                        # fc weight panels per core
PANW = VSLICE // NPANEL           # 500 cols per panel (one PSUM chunk)
MT = HIDDEN // P                  # 4 hidden tiles
NTOK = B * T                      # 8192
NG = NTOK // P                    # 64 gathers per direction
CHTOK = 512                       # tokens per prologue chunk
NCH = NTOK // CHTOK               # 16 chunks
STEPS_PER_CH = CHTOK // B         # 32 steps of precomp per chunk
SLOT = 16                         # columns per step per m-block
MBLK = 513 * SLOT                 # hbig columns per m-block (8208)
PRE_BLK = 32                      # scan-time precomp streaming block (steps)
BF = mybir.dt.bfloat16
F32 = mybir.dt.float32

_CACHED_NC = None


def build():
    nc = bacc.Bacc(None, target_bir_lowering=False, debug=False)

    emb = nc.declare_dram_parameter("emb", [VOCAB, EMBED], F32, isOutput=False)
    ids_in = {d: nc.declare_dram_parameter(f"ids_{d}", [P, NG], mybir.dt.int32,
                                           isOutput=False) for d in "fb"}
    whh_in = {d: nc.declare_dram_parameter(f"whh_{d}", [HIDDEN, HIDDEN], F32,
                                           isOutput=False) for d in "fb"}
    wxh_in = {d: nc.declare_dram_parameter(f"wxh_{d}", [EMBED, HIDDEN], F32,
                                           isOutput=False) for d in "fb"}
    bh_in = {d: nc.declare_dram_parameter(f"bh_{d}", [1, HIDDEN], F32,
                                          isOutput=False) for d in "fb"}
    h0_in = nc.declare_dram_parameter("h0", [P, MT * B], F32, isOutput=False)
    wfc_in = nc.declare_dram_parameter("wfc", [2 * HIDDEN, VSLICE], F32,
                                       isOutput=False)
    out = nc.declare_dram_parameter("out", [NTOK, VSLICE], F32, isOutput=True)

    pre_dram = {d: nc.dram_tensor(f"pre_{d}", [P, T * MT * B], BF)
                for d in "fb"}

    from contextlib import ExitStack
    with tile.TileContext(nc) as tc:
        with tc.tile_pool(name="const", bufs=1) as const, \
             tc.tile_pool(name="hpool", bufs=1) as hpool, \
             tc.tile_pool(name="ps", bufs=2, space="PSUM") as ps:
            stackA = ExitStack()
            stage = stackA.enter_context(tc.tile_pool(name="stage", bufs=2))
            gat = stackA.enter_context(tc.tile_pool(name="gat", bufs=2))
            xtp = stackA.enter_context(tc.tile_pool(name="xt", bufs=2))
            prest = stackA.enter_context(tc.tile_pool(name="prest", bufs=2))

            # ---------------- constants ----------------
            ident_f = const.tile([P, P], F32, tag="ident_f")
            make_identity(nc, ident_f[:])
            ident_b = const.tile([P, P], BF, tag="ident_b")
            nc.vector.tensor_copy(out=ident_b[:], in_=ident_f[:])
            ones_row = const.tile([1, CHTOK], BF, tag="ones_row")
            nc.gpsimd.memset(ones_row[:], 1.0)

            whh = {}
            wxh = {}
            bh = {}
            ids_sb = {}
            for d in "fb":
                for kt in range(MT):
                    wf = stage.tile([P, HIDDEN], F32, tag="wstage", name="wf")
                    nc.sync.dma_start(out=wf[:],
                                      in_=whh_in[d][kt * P:(kt + 1) * P, :])
                    for mt in range(MT):
                        wc = const.tile([P, P], BF, tag=f"whh{d}{kt}{mt}",
                                        name="wc")
                        nc.vector.tensor_copy(
                            out=wc[:], in_=wf[:, mt * P:(mt + 1) * P])
                        whh[(d, kt, mt)] = wc
                for e in range(EMBED // P):
                    wf2 = stage.tile([P, HIDDEN], F32, tag="wstage", name="wf2")
                    nc.sync.dma_start(out=wf2[:],
                                      in_=wxh_in[d][e * P:(e + 1) * P, :])
                    for mt in range(MT):
                        wc2 = const.tile([P, P], BF, tag=f"wxh{d}{e}{mt}",
                                         name="wc2")
                        nc.vector.tensor_copy(
                            out=wc2[:], in_=wf2[:, mt * P:(mt + 1) * P])
                        wxh[(d, e, mt)] = wc2
                btf = stage.tile([1, HIDDEN], F32, tag="bstage", name="btf")
                nc.sync.dma_start(out=btf[:], in_=bh_in[d][:, :])
                bt = const.tile([1, HIDDEN], BF, tag=f"bh{d}", name="bt")
                nc.vector.tensor_copy(out=bt[:], in_=btf[:])
                bh[d] = bt
                it_ = const.tile([P, NG], mybir.dt.int32, tag=f"ids{d}",
                                 name="it_")
                nc.sync.dma_start(out=it_[:], in_=ids_in[d][:, :])
                ids_sb[d] = it_

            h0f = const.tile([P, MT * B], F32, tag="h0f")
            nc.sync.dma_start(out=h0f[:], in_=h0_in[:, :])

            # hbig: column = m*MBLK + slot*16 + b
            hbig = {d: hpool.tile([P, MT * MBLK], BF, tag=f"hbig{d}",
                                  name=f"hbig{d}") for d in "fb"}

            def hslot_w(d, slot):
                # strided AP: 4 m-blocks x 16 cols at given slot (write/tanh)
                return hbig[d][:].rearrange(
                    "p (m s) -> p m s", m=MT)[:, :, slot * SLOT:(slot + 1) * SLOT]

            def hslot_r(d, slot, kt):
                # one m-block's 16 cols (matmul rhs)
                base = kt * MBLK + slot * SLOT
                return hbig[d][:, base:base + SLOT]

            # h0 into fwd slot 0 and bwd slot 512 (packed -> strided m-blocks)
            for d, slot in (("f", 0), ("b", T)):
                nc.vector.tensor_copy(
                    out=hslot_w(d, slot),
                    in_=h0f[:].rearrange("p (m s) -> p m s", m=MT))

            # ---------------- phase A: gather + x-projection ----------------
            evict_flip = [0]

            def evict_engine():
                evict_flip[0] ^= 1
                return nc.vector if evict_flip[0] else nc.scalar

            for c in range(NCH):
                for d in "fb":
                    xt = {e: xtp.tile([P, CHTOK], BF, tag=f"xt{e}",
                                      name=f"xt{e}")
                          for e in range(EMBED // P)}
                    for g in range(CHTOK // P):
                        gi = c * (CHTOK // P) + g
                        xg = gat.tile([P, EMBED], F32, tag="xg", name="xg")
                        nc.gpsimd.indirect_dma_start(
                            out=xg[:], out_offset=None, in_=emb[:],
                            in_offset=bass.IndirectOffsetOnAxis(
                                ap=ids_sb[d][:, gi:gi + 1], axis=0),
                        )
                        for e in range(EMBED // P):
                            tp = ps.tile([P, P], F32, tag="big0", name="tp")
                            nc.tensor.transpose(
                                out=tp[:], in_=xg[:, e * P:(e + 1) * P],
                                identity=ident_f[:])
                            nc.vector.tensor_copy(
                                out=xt[e][:, g * P:(g + 1) * P], in_=tp[:])
                    # project: pre[m] = sum_e wxh[e,m]^T @ xt[e]  (+ bias)
                    stg = prest.tile([P, STEPS_PER_CH * MT * B], BF,
                                     tag="prestg", name="stg")
                    stg3 = stg[:].rearrange("p (s m) -> p s m", m=MT * B)
                    for mt in range(MT):
                        zp = ps.tile([P, CHTOK], F32, tag=f"big{mt % 2}",
                                     name="zp")
                        for e in range(EMBED // P):
                            nc.tensor.matmul(
                                out=zp[:], lhsT=wxh[(d, e, mt)][:],
                                rhs=xt[e][:],
                                start=(e == 0), stop=False,
                                skip_group_check=True)
                        # bias: [1,128] b-row (stationary) x [1,CHTOK] ones
                        nc.tensor.matmul(
                            out=zp[:], lhsT=bh[d][:, mt * P:(mt + 1) * P],
                            rhs=ones_row[:], start=False, stop=True,
                            skip_group_check=True)
                        # pack-write (strided out), bf16 on write
                        dst = stg3[:, :, mt * B:(mt + 1) * B]
                        eng = evict_engine()
                        if eng is nc.scalar:
                            nc.scalar.activation(
                                out=dst, in_=zp[:],
                                func=mybir.ActivationFunctionType.Copy)
                        else:
                            nc.vector.tensor_copy(out=dst, in_=zp[:])
                    nc.sync.dma_start(
                        out=pre_dram[d][:, c * (STEPS_PER_CH * 64):
                                        (c + 1) * (STEPS_PER_CH * 64)],
                        in_=stg[:])

            stackA.close()

            # ---------------- phase B: the two scans, interleaved ----------
            stackB = ExitStack()
            prebp = stackB.enter_context(tc.tile_pool(name="prebp", bufs=2))
            preb = {}

            def pre_tile(d, blk):
                t_ = prebp.tile([P, PRE_BLK * 64], BF, tag=f"preb{d}",
                                name="t_")
                nc.gpsimd.dma_start(
                    out=t_[:],
                    in_=pre_dram[d][:, blk * (PRE_BLK * 64):
                                    (blk + 1) * (PRE_BLK * 64)])
                return t_

            for s in range(T):
                for d in "fb":
                    if s % PRE_BLK == 0:
                        preb[d] = pre_tile(d, s // PRE_BLK)
                    slot_in = s if d == "f" else T - s
                    slot_out = s + 1 if d == "f" else T - 1 - s
                    z = ps.tile([P, MT * B], F32, tag=f"z{d}", name="z")
                    nc.tensor.matmul(
                        out=z[:], lhsT=ident_b[:],
                        rhs=preb[d][:, (s % PRE_BLK) * 64:
                                    (s % PRE_BLK) * 64 + 64],
                        start=True, stop=False, skip_group_check=True)
                    for mt in range(MT):
                        for kt in range(MT):
                            nc.tensor.matmul(
                                out=z[:, mt * B:(mt + 1) * B],
                                lhsT=whh[(d, kt, mt)][:],
                                rhs=hslot_r(d, slot_in, kt),
                                start=False,
                                stop=(mt == MT - 1 and kt == MT - 1),
                                skip_group_check=True)
                    nc.scalar.activation(
                        out=hslot_w(d, slot_out), in_=z[:],
                        func=mybir.ActivationFunctionType.Tanh)

            stackB.close()
            stackC = ExitStack()
            wstg = stackC.enter_context(tc.tile_pool(name="wstg", bufs=2))
            wfcp = stackC.enter_context(tc.tile_pool(name="wfcp", bufs=1))
            evp = stackC.enter_context(tc.tile_pool(name="evp", bufs=1))

            # ---------------- phase C: fc projection ----------------
            # middle-out M-tile order matches h availability (fwd fills
            # left-to-right, bwd right-to-left; both meet in the middle)
            n_mt = NTOK // P  # 64
            order = []
            lo, hi = n_mt // 2 - 1, n_mt // 2
            while hi < n_mt:
                order.append(hi)
                order.append(lo)
                hi += 1
                lo -= 1

            for v in range(NPANEL):
                wfc = {}
                for kt in range(2 * HIDDEN // P):
                    wfs = wstg.tile([P, PANW], F32, tag="wfcstage", name="wfs")
                    nc.gpsimd.dma_start(
                        out=wfs[:],
                        in_=wfc_in[kt * P:(kt + 1) * P,
                                   v * PANW:(v + 1) * PANW])
                    wfb = wfcp.tile([P, PANW], BF, tag=f"wfc{kt}", name="wfb")
                    nc.vector.tensor_copy(out=wfb[:], in_=wfs[:])
                    wfc[kt] = wfb
                for mt in order:
                    t0 = mt * 8
                    z = ps.tile([P, PANW], F32, tag=f"big{mt % 2}", name="z")
                    for kt in range(2 * HIDDEN // P):
                        if kt < MT:
                            lhsT = hbig["f"][:, kt * MBLK + (t0 + 1) * SLOT:
                                             kt * MBLK + (t0 + 1) * SLOT + P]
                        else:
                            lhsT = hbig["b"][:, (kt - MT) * MBLK + t0 * SLOT:
                                             (kt - MT) * MBLK + t0 * SLOT + P]
                        nc.tensor.matmul(out=z[:], lhsT=lhsT, rhs=wfc[kt][:],
                                         start=(kt == 0),
                                         stop=(kt == 2 * HIDDEN // P - 1))
                    ev = evp.tile([P, PANW], F32, tag=f"ev{mt % 4}", name="ev")
                    eng = evict_engine()
                    if eng is nc.scalar:
                        nc.scalar.activation(
                            out=ev[:], in_=z[:],
                            func=mybir.ActivationFunctionType.Copy)
                    else:
                        nc.vector.tensor_copy(out=ev[:], in_=z[:])
                    nc.sync.dma_start(
                        out=out[mt * P:(mt + 1) * P, v * PANW:(v + 1) * PANW],
                        in_=ev[:])
            stackC.close()
    nc.finalize()
    return nc


def _pack_h(hT):
    # [H, B] -> [128, MT*B] packed (col = m*16+b)
    return np.ascontiguousarray(
        hT.reshape(MT, P, B).transpose(1, 0, 2).reshape(P, MT * B))


def kernel(inputs, h_prev, emb, W_xh_f, W_hh_f, b_h_f,
           W_xh_b, W_hh_b, b_h_b, W_fc, b_fc):
    global _CACHED_NC
    inputs = np.asarray(inputs, dtype=np.int32)
    h_prev = np.asarray(h_prev, dtype=np.float32)
    emb = np.ascontiguousarray(np.asarray(emb, dtype=np.float32))
    W_xh = {"f": np.asarray(W_xh_f, np.float32),
            "b": np.asarray(W_xh_b, np.float32)}
    W_hh = {"f": np.asarray(W_hh_f, np.float32),
            "b": np.asarray(W_hh_b, np.float32)}
    b_h = {"f": np.asarray(b_h_f, np.float32),
           "b": np.asarray(b_h_b, np.float32)}
    W_fc = np.asarray(W_fc, np.float32)
    b_fc = np.asarray(b_fc, np.float32)

    if _CACHED_NC is None:
        _CACHED_NC = build()
    nc = _CACHED_NC

    # token order (t, b); bwd uses time-reversed ids
    ids_f = inputs.T.reshape(NG, P).T  # [128, 64] col g = tokens g*128..
    ids_b = inputs[:, ::-1].T.reshape(NG, P).T
    h0 = _pack_h(h_prev.T)

    base = {
        "emb": emb,
        "ids_f": np.ascontiguousarray(ids_f),
        "ids_b": np.ascontiguousarray(ids_b),
        "h0": h0,
    }
    for d in "fb":
        base[f"whh_{d}"] = W_hh[d]
        base[f"wxh_{d}"] = W_xh[d]
        base[f"bh_{d}"] = np.ascontiguousarray(
            b_h[d].reshape(1, HIDDEN))

    in_maps = []
    for c in range(NCORES):
        m = dict(base)
        m["wfc"] = np.ascontiguousarray(W_fc[:, c * VSLICE:(c + 1) * VSLICE])
        in_maps.append(m)

    res = run_bass_kernel_spmd(nc, in_maps, core_ids=list(range(NCORES)))
    slabs = [res.results[c]["out"] for c in range(NCORES)]
    full = np.concatenate(slabs, axis=1)              # [8192, 32000] (t,b)
    full = full.reshape(T, B, VOCAB).transpose(1, 0, 2)
    return np.ascontiguousarray(full + b_fc)


# revision 7
# speedup vs baseline: 1.1372x; 1.1372x over previous
"""Bidirectional RNN (embed -> fwd/bwd tanh scans -> vocab projection) on 8
TRN2 NeuronCores.

Strategy (per core, SPMD, no collectives):
  - Every core runs BOTH recurrent scans redundantly for the full batch (the
    scan is serial in T and cheap relative to the fc projection; redundancy
    avoids any inter-core exchange of hidden states).
  - The fc projection + embedding table rows are consumed identically on all
    cores, but the big fc weight is column-split over vocab: core c receives
    W_fc[:, c*4000:(c+1)*4000] through its in_map and writes the matching
    output slab. Rank never appears in the program.
  - Host-side numpy does only data movement: token reordering, packing
    transposes, vocab slicing, and the (zero) b_fc broadcast add.

Layouts:
  - "packed" hidden state column block per step: col = m*16 + b for hidden
    row m*128+p  (m: 4 H-tiles, b: batch 16) -> one PSUM bank [128, 64].
  - hbig (per dir) [128, 4*513*16]: column m*8208 + slot*16 + b. Slots are
    original time indices; fwd fills slot t+1 left-to-right, bwd fills slot t
    right-to-left. fc reads 128-column contiguous runs as matmul weights.
"""
import numpy as np

import concourse.bacc as bacc
import concourse.bass as bass
import concourse.mybir as mybir
import concourse.tile as tile
from concourse.bass_utils import run_bass_kernel_spmd
from concourse.masks import make_identity

P = 128
VOCAB, EMBED, HIDDEN = 32000, 256, 512
B, T = 16, 512
NCORES = 8
VSLICE = VOCAB // NCORES          # 4000 vocab cols per core
NPANEL = 8                        # fc weight panels per core
PANW = VSLICE // NPANEL           # 500 cols per panel (one PSUM chunk)
MT = HIDDEN // P                  # 4 hidden tiles
NTOK = B * T                      # 8192
NG = NTOK // P                    # 64 gathers per direction
CHTOK = 512                       # tokens per prologue chunk
NCH = NTOK // CHTOK               # 16 chunks
STEPS_PER_CH = CHTOK // B         # 32 steps of precomp per chunk
SLOT = 16                         # columns per step per m-block
MBLK = 513 * SLOT                 # hbig columns per m-block (8208)
PRE_BLK = 32                      # scan-time precomp streaming block (steps)
BF = mybir.dt.bfloat16
F32 = mybir.dt.float32

_CACHED_NC = None


def build():
    nc = bacc.Bacc(None, target_bir_lowering=False, debug=False)

    emb = nc.declare_dram_parameter("emb", [VOCAB, EMBED], F32, isOutput=False)
    ids_in = {d: nc.declare_dram_parameter(f"ids_{d}", [P, NG], mybir.dt.int32,
                                           isOutput=False) for d in "fb"}
    whh_in = {d: nc.declare_dram_parameter(f"whh_{d}", [HIDDEN, HIDDEN], F32,
                                           isOutput=False) for d in "fb"}
    wxh_in = {d: nc.declare_dram_parameter(f"wxh_{d}", [EMBED, HIDDEN], F32,
                                           isOutput=False) for d in "fb"}
    bh_in = {d: nc.declare_dram_parameter(f"bh_{d}", [1, HIDDEN], F32,
                                          isOutput=False) for d in "fb"}
    h0_in = nc.declare_dram_parameter("h0", [P, MT * B], F32, isOutput=False)
    wfc_in = nc.declare_dram_parameter("wfc", [2 * HIDDEN, VSLICE], F32,
                                       isOutput=False)
    out = nc.declare_dram_parameter("out", [NTOK, VSLICE], F32, isOutput=True)



    from contextlib import ExitStack
    with tile.TileContext(nc) as tc:
        with tc.tile_pool(name="const", bufs=1) as const, \
             tc.tile_pool(name="hpool", bufs=1) as hpool, \
             tc.tile_pool(name="ps", bufs=2, space="PSUM") as ps:
            stackA = ExitStack()
            stage = stackA.enter_context(tc.tile_pool(name="stage", bufs=2))
            gat = stackA.enter_context(tc.tile_pool(name="gat", bufs=2))
            xtp = stackA.enter_context(tc.tile_pool(name="xt", bufs=2))
            prest = stackA.enter_context(tc.tile_pool(name="prest", bufs=3))

            # ---------------- constants ----------------
            ident_f = const.tile([P, P], F32, tag="ident_f")
            make_identity(nc, ident_f[:])
            ident_b = const.tile([P, P], BF, tag="ident_b")
            nc.vector.tensor_copy(out=ident_b[:], in_=ident_f[:])
            ones_row = const.tile([1, CHTOK], BF, tag="ones_row")
            nc.gpsimd.memset(ones_row[:], 1.0)

            whh = {}
            wxh = {}
            bh = {}
            ids_sb = {}
            for d in "fb":
                for kt in range(MT):
                    wf = stage.tile([P, HIDDEN], F32, tag="wstage", name="wf")
                    nc.sync.dma_start(out=wf[:],
                                      in_=whh_in[d][kt * P:(kt + 1) * P, :])
                    for mt in range(MT):
                        wc = const.tile([P, P], BF, tag=f"whh{d}{kt}{mt}",
                                        name="wc")
                        nc.vector.tensor_copy(
                            out=wc[:], in_=wf[:, mt * P:(mt + 1) * P])
                        whh[(d, kt, mt)] = wc
                for e in range(EMBED // P):
                    wf2 = stage.tile([P, HIDDEN], F32, tag="wstage", name="wf2")
                    nc.sync.dma_start(out=wf2[:],
                                      in_=wxh_in[d][e * P:(e + 1) * P, :])
                    for mt in range(MT):
                        wc2 = const.tile([P, P], BF, tag=f"wxh{d}{e}{mt}",
                                         name="wc2")
                        nc.vector.tensor_copy(
                            out=wc2[:], in_=wf2[:, mt * P:(mt + 1) * P])
                        wxh[(d, e, mt)] = wc2
                btf = stage.tile([1, HIDDEN], F32, tag="bstage", name="btf")
                nc.sync.dma_start(out=btf[:], in_=bh_in[d][:, :])
                bt = const.tile([1, HIDDEN], BF, tag=f"bh{d}", name="bt")
                nc.vector.tensor_copy(out=bt[:], in_=btf[:])
                bh[d] = bt
                it_ = const.tile([P, NG], mybir.dt.int32, tag=f"ids{d}",
                                 name="it_")
                nc.sync.dma_start(out=it_[:], in_=ids_in[d][:, :])
                ids_sb[d] = it_

            h0f = const.tile([P, MT * B], F32, tag="h0f")
            nc.sync.dma_start(out=h0f[:], in_=h0_in[:, :])

            # hbig: column = m*MBLK + slot*16 + b
            hbig = {d: hpool.tile([P, MT * MBLK], BF, tag=f"hbig{d}",
                                  name=f"hbig{d}") for d in "fb"}

            def hslot_w(d, slot):
                # strided AP: 4 m-blocks x 16 cols at given slot (write/tanh)
                return hbig[d][:].rearrange(
                    "p (m s) -> p m s", m=MT)[:, :, slot * SLOT:(slot + 1) * SLOT]

            def hslot_r(d, slot, kt):
                # one m-block's 16 cols (matmul rhs)
                base = kt * MBLK + slot * SLOT
                return hbig[d][:, base:base + SLOT]

            # h0 into fwd slot 0 and bwd slot 512 (packed -> strided m-blocks)
            for d, slot in (("f", 0), ("b", T)):
                nc.vector.tensor_copy(
                    out=hslot_w(d, slot),
                    in_=h0f[:].rearrange("p (m s) -> p m s", m=MT))

            # ---------------- phase A: gather + x-projection ----------------
            evict_flip = [0]

            def evict_engine():
                evict_flip[0] ^= 1
                return nc.vector if evict_flip[0] else nc.scalar

            stg_cur = {}

            def emit_chunk(c, d):
                xt = {e: xtp.tile([P, CHTOK], BF, tag=f"xt{e}",
                                  name=f"xt{e}")
                      for e in range(EMBED // P)}
                for g in range(CHTOK // P):
                    gi = c * (CHTOK // P) + g
                    xg = gat.tile([P, EMBED], F32, tag="xg", name="xg")
                    nc.gpsimd.indirect_dma_start(
                        out=xg[:], out_offset=None, in_=emb[:],
                        in_offset=bass.IndirectOffsetOnAxis(
                            ap=ids_sb[d][:, gi:gi + 1], axis=0),
                    )
                    for e in range(EMBED // P):
                        tp = ps.tile([P, P], F32, tag="big0", name="tp")
                        nc.tensor.transpose(
                            out=tp[:], in_=xg[:, e * P:(e + 1) * P],
                            identity=ident_f[:])
                        nc.vector.tensor_copy(
                            out=xt[e][:, g * P:(g + 1) * P], in_=tp[:])
                stg = prest.tile([P, STEPS_PER_CH * MT * B], BF,
                                 tag=f"prestg{d}", name="stg")
                stg3 = stg[:].rearrange("p (s m) -> p s m", m=MT * B)
                for mt in range(MT):
                    zp = ps.tile([P, CHTOK], F32, tag=f"big{mt % 2}",
                                 name="zp")
                    for e in range(EMBED // P):
                        nc.tensor.matmul(
                            out=zp[:], lhsT=wxh[(d, e, mt)][:],
                            rhs=xt[e][:],
                            start=(e == 0), stop=False,
                            skip_group_check=True)
                    nc.tensor.matmul(
                        out=zp[:], lhsT=bh[d][:, mt * P:(mt + 1) * P],
                        rhs=ones_row[:], start=False, stop=True,
                        skip_group_check=True)
                    dst = stg3[:, :, mt * B:(mt + 1) * B]
                    eng = evict_engine()
                    if eng is nc.scalar:
                        nc.scalar.activation(
                            out=dst, in_=zp[:],
                            func=mybir.ActivationFunctionType.Copy)
                    else:
                        nc.vector.tensor_copy(out=dst, in_=zp[:])
                stg_cur[d] = stg

            for c in range(NCH):
                for d in "fb":
                    emit_chunk(c, d)
                for s in range(c * STEPS_PER_CH, (c + 1) * STEPS_PER_CH):
                    for d in "fb":
                        slot_in = s if d == "f" else T - s
                        slot_out = s + 1 if d == "f" else T - 1 - s
                        z = ps.tile([P, MT * B], F32, tag=f"z{d}", name="z")
                        nc.tensor.matmul(
                            out=z[:], lhsT=ident_b[:],
                            rhs=stg_cur[d][:, (s % STEPS_PER_CH) * 64:
                                           (s % STEPS_PER_CH) * 64 + 64],
                            start=True, stop=False, skip_group_check=True)
                        for mt in range(MT):
                            for kt in range(MT):
                                nc.tensor.matmul(
                                    out=z[:, mt * B:(mt + 1) * B],
                                    lhsT=whh[(d, kt, mt)][:],
                                    rhs=hslot_r(d, slot_in, kt),
                                    start=False,
                                    stop=(mt == MT - 1 and kt == MT - 1),
                                    skip_group_check=True)
                        nc.scalar.activation(
                            out=hslot_w(d, slot_out), in_=z[:],
                            func=mybir.ActivationFunctionType.Tanh)

            stackA.close()
            stackC = ExitStack()
            wstg = stackC.enter_context(tc.tile_pool(name="wstg", bufs=2))
            wfcp = stackC.enter_context(tc.tile_pool(name="wfcp", bufs=1))
            evp = stackC.enter_context(tc.tile_pool(name="evp", bufs=1))

            # ---------------- phase C: fc projection ----------------
            # middle-out M-tile order matches h availability (fwd fills
            # left-to-right, bwd right-to-left; both meet in the middle)
            n_mt = NTOK // P  # 64
            order = []
            lo, hi = n_mt // 2 - 1, n_mt // 2
            while hi < n_mt:
                order.append(hi)
                order.append(lo)
                hi += 1
                lo -= 1

            for v in range(NPANEL):
                wfc = {}
                for kt in range(2 * HIDDEN // P):
                    wfs = wstg.tile([P, PANW], F32, tag="wfcstage", name="wfs")
                    nc.sync.dma_start(
                        out=wfs[:],
                        in_=wfc_in[kt * P:(kt + 1) * P,
                                   v * PANW:(v + 1) * PANW])
                    wfb = wfcp.tile([P, PANW], BF, tag=f"wfc{kt}", name="wfb")
                    nc.vector.tensor_copy(out=wfb[:], in_=wfs[:])
                    wfc[kt] = wfb
                for mt in order:
                    t0 = mt * 8
                    z = ps.tile([P, PANW], F32, tag=f"big{mt % 2}", name="z")
                    for kt in range(2 * HIDDEN // P):
                        if kt < MT:
                            lhsT = hbig["f"][:, kt * MBLK + (t0 + 1) * SLOT:
                                             kt * MBLK + (t0 + 1) * SLOT + P]
                        else:
                            lhsT = hbig["b"][:, (kt - MT) * MBLK + t0 * SLOT:
                                             (kt - MT) * MBLK + t0 * SLOT + P]
                        nc.tensor.matmul(out=z[:], lhsT=lhsT, rhs=wfc[kt][:],
                                         start=(kt == 0),
                                         stop=(kt == 2 * HIDDEN // P - 1))
                    ev = evp.tile([P, PANW], F32, tag=f"ev{mt % 4}", name="ev")
                    eng = evict_engine()
                    if eng is nc.scalar:
                        nc.scalar.activation(
                            out=ev[:], in_=z[:],
                            func=mybir.ActivationFunctionType.Copy)
                    else:
                        nc.vector.tensor_copy(out=ev[:], in_=z[:])
                    nc.sync.dma_start(
                        out=out[mt * P:(mt + 1) * P, v * PANW:(v + 1) * PANW],
                        in_=ev[:])
            stackC.close()
    nc.finalize()
    return nc


def _pack_h(hT):
    # [H, B] -> [128, MT*B] packed (col = m*16+b)
    return np.ascontiguousarray(
        hT.reshape(MT, P, B).transpose(1, 0, 2).reshape(P, MT * B))


def kernel(inputs, h_prev, emb, W_xh_f, W_hh_f, b_h_f,
           W_xh_b, W_hh_b, b_h_b, W_fc, b_fc):
    global _CACHED_NC
    inputs = np.asarray(inputs, dtype=np.int32)
    h_prev = np.asarray(h_prev, dtype=np.float32)
    emb = np.ascontiguousarray(np.asarray(emb, dtype=np.float32))
    W_xh = {"f": np.asarray(W_xh_f, np.float32),
            "b": np.asarray(W_xh_b, np.float32)}
    W_hh = {"f": np.asarray(W_hh_f, np.float32),
            "b": np.asarray(W_hh_b, np.float32)}
    b_h = {"f": np.asarray(b_h_f, np.float32),
           "b": np.asarray(b_h_b, np.float32)}
    W_fc = np.asarray(W_fc, np.float32)
    b_fc = np.asarray(b_fc, np.float32)

    if _CACHED_NC is None:
        _CACHED_NC = build()
    nc = _CACHED_NC

    # token order (t, b); bwd uses time-reversed ids
    ids_f = inputs.T.reshape(NG, P).T  # [128, 64] col g = tokens g*128..
    ids_b = inputs[:, ::-1].T.reshape(NG, P).T
    h0 = _pack_h(h_prev.T)

    base = {
        "emb": emb,
        "ids_f": np.ascontiguousarray(ids_f),
        "ids_b": np.ascontiguousarray(ids_b),
        "h0": h0,
    }
    for d in "fb":
        base[f"whh_{d}"] = W_hh[d]
        base[f"wxh_{d}"] = W_xh[d]
        base[f"bh_{d}"] = np.ascontiguousarray(
            b_h[d].reshape(1, HIDDEN))

    in_maps = []
    for c in range(NCORES):
        m = dict(base)
        m["wfc"] = np.ascontiguousarray(W_fc[:, c * VSLICE:(c + 1) * VSLICE])
        in_maps.append(m)

    res = run_bass_kernel_spmd(nc, in_maps, core_ids=list(range(NCORES)))
    slabs = [res.results[c]["out"] for c in range(NCORES)]
    full = np.concatenate(slabs, axis=1)              # [8192, 32000] (t,b)
    full = full.reshape(T, B, VOCAB).transpose(1, 0, 2)
    return np.ascontiguousarray(full + b_fc)


# revision 8
# speedup vs baseline: 1.2995x; 1.1427x over previous
"""Bidirectional RNN (embed -> fwd/bwd tanh scans -> vocab projection) on 8
TRN2 NeuronCores.

Strategy (per core, SPMD, identical program, no collectives):
  - Direction-split data parallelism: cores 0-3 run the FORWARD scan, cores
    4-7 the BACKWARD scan. The direction is chosen purely by the per-core
    input data (reversed token order + that direction's weights), so the
    instruction stream is identical on all cores.
  - The fc projection is split along BOTH vocab (4 column slices of 8000)
    and the contraction (each core uses only its own direction's 512 rows of
    W_fc). Core c and core c+4 produce additive partials for the same vocab
    slice; the host sums them (plus b_fc). This halves per-core scan work
    and removes any cross-direction dependency, so fc matmuls overlap the
    scan's latency bubbles.
  - Embedding gather is an indirect DMA per 128 tokens; gathered rows are
    PE-transposed and projected (x @ W_xh + b_h) chunk-by-chunk, with the
    scan consuming each chunk's staging tile directly from SBUF.

Layouts:
  - packed hidden-state step block: column m*16 + b for hidden row m*128+p
    (m: 4 H-tiles, b: batch 16) -> one PSUM bank [128, 64].
  - h ring [128, 4*513*16]: column m*8208 + slot*16 + b. Slot s+1 holds the
    state after scan step s (scan order). fc reads 128-column contiguous
    runs as matmul stationary operands. Backward cores' output rows come
    out time-reversed; the host flips them.
"""
import numpy as np

import concourse.bacc as bacc
import concourse.bass as bass
import concourse.mybir as mybir
import concourse.tile as tile
from concourse.bass_utils import run_bass_kernel_spmd
from concourse.masks import make_identity

P = 128
VOCAB, EMBED, HIDDEN = 32000, 256, 512
B, T = 16, 512
NCORES = 8
VSLICE = VOCAB // 4               # 8000 vocab cols per core (pairs share)
PANW = 500                        # cols per PSUM chunk
NCHUNK_V = VSLICE // PANW         # 16
MT = HIDDEN // P                  # 4 hidden tiles
ET = EMBED // P                   # 2 embed tiles
NTOK = B * T                      # 8192
NG = NTOK // P                    # 64 gathers
CHTOK = 512                       # tokens per prologue chunk
NCH = NTOK // CHTOK               # 16 chunks
SPC = CHTOK // B                  # 32 steps per chunk
SLOT = 16
MBLK = (T + 1) * SLOT             # 8208 h-ring cols per m-block
BF = mybir.dt.bfloat16
F32 = mybir.dt.float32

_CACHED_NC = None


def build():
    nc = bacc.Bacc(None, target_bir_lowering=False, debug=False)

    emb = nc.declare_dram_parameter("emb", [VOCAB, EMBED], F32, isOutput=False)
    ids_in = nc.declare_dram_parameter("ids_a", [P, NG], mybir.dt.int32,
                                       isOutput=False)
    whh_in = nc.declare_dram_parameter("whh_a", [HIDDEN, HIDDEN], F32,
                                       isOutput=False)
    wxh_in = nc.declare_dram_parameter("wxh_a", [EMBED, HIDDEN], F32,
                                       isOutput=False)
    bh_in = nc.declare_dram_parameter("bh_a", [1, HIDDEN], F32, isOutput=False)
    h0_in = nc.declare_dram_parameter("h0", [P, MT * B], F32, isOutput=False)
    wfc_in = nc.declare_dram_parameter("wfc_a", [HIDDEN, VSLICE], F32,
                                       isOutput=False)
    out = nc.declare_dram_parameter("out", [NTOK, VSLICE], F32, isOutput=True)

    from contextlib import ExitStack
    with tile.TileContext(nc) as tc:
        with tc.tile_pool(name="const", bufs=1) as const, \
             tc.tile_pool(name="hpool", bufs=1) as hpool, \
             tc.tile_pool(name="wfcp", bufs=1) as wfcp, \
             tc.tile_pool(name="evp", bufs=1) as evp, \
             tc.tile_pool(name="ps", bufs=2, space="PSUM") as ps:
            stackA = ExitStack()
            stage = stackA.enter_context(tc.tile_pool(name="stage", bufs=2))
            gat = stackA.enter_context(tc.tile_pool(name="gat", bufs=2))
            xtp = stackA.enter_context(tc.tile_pool(name="xt", bufs=2))
            prest = stackA.enter_context(tc.tile_pool(name="prest", bufs=3))

            # ---------------- constants ----------------
            ident_f = const.tile([P, P], F32, tag="ident_f")
            make_identity(nc, ident_f[:])
            ident_b = const.tile([P, P], BF, tag="ident_b")
            nc.vector.tensor_copy(out=ident_b[:], in_=ident_f[:])
            ones_row = const.tile([1, CHTOK], BF, tag="ones_row")
            nc.gpsimd.memset(ones_row[:], 1.0)

            whh = {}
            for kt in range(MT):
                wf = stage.tile([P, HIDDEN], F32, tag="wstage", name="wf")
                nc.sync.dma_start(out=wf[:], in_=whh_in[kt * P:(kt + 1) * P, :])
                for mt in range(MT):
                    wc = const.tile([P, P], BF, tag=f"whh{kt}{mt}", name="wc")
                    nc.vector.tensor_copy(out=wc[:],
                                          in_=wf[:, mt * P:(mt + 1) * P])
                    whh[(kt, mt)] = wc
            wxh = {}
            for e in range(ET):
                wf2 = stage.tile([P, HIDDEN], F32, tag="wstage", name="wf2")
                nc.sync.dma_start(out=wf2[:], in_=wxh_in[e * P:(e + 1) * P, :])
                for mt in range(MT):
                    wc2 = const.tile([P, P], BF, tag=f"wxh{e}{mt}", name="wc2")
                    nc.vector.tensor_copy(out=wc2[:],
                                          in_=wf2[:, mt * P:(mt + 1) * P])
                    wxh[(e, mt)] = wc2
            btf = stage.tile([1, HIDDEN], F32, tag="bstage", name="btf")
            nc.sync.dma_start(out=btf[:], in_=bh_in[:, :])
            bh = const.tile([1, HIDDEN], BF, tag="bh", name="bh")
            nc.vector.tensor_copy(out=bh[:], in_=btf[:])
            ids_sb = const.tile([P, NG], mybir.dt.int32, tag="ids", name="ids")
            nc.sync.dma_start(out=ids_sb[:], in_=ids_in[:, :])
            h0f = const.tile([P, MT * B], F32, tag="h0f")
            nc.sync.dma_start(out=h0f[:], in_=h0_in[:, :])

            # W_fc resident: 4 k-tiles [128, VSLICE] bf16, converted in
            # column chunks through a small f32 staging tile
            wfc = {}
            for kt in range(MT):
                wfb = wfcp.tile([P, VSLICE], BF, tag=f"wfc{kt}", name="wfb")
                wfc[kt] = wfb
            for kt in range(MT):
                for q in range(4):
                    qw = VSLICE // 4
                    wfs = stage.tile([P, qw], F32, tag="wfcstage", name="wfs")
                    nc.sync.dma_start(
                        out=wfs[:],
                        in_=wfc_in[kt * P:(kt + 1) * P, q * qw:(q + 1) * qw])
                    nc.vector.tensor_copy(out=wfc[kt][:, q * qw:(q + 1) * qw],
                                          in_=wfs[:])

            # h ring
            hbig = hpool.tile([P, MT * MBLK], BF, tag="hbig", name="hbig")

            def hslot_w(slot):
                return hbig[:].rearrange(
                    "p (m s) -> p m s", m=MT)[:, :, slot * SLOT:(slot + 1) * SLOT]

            def hslot_r(slot, kt):
                base = kt * MBLK + slot * SLOT
                return hbig[:, base:base + SLOT]

            nc.vector.tensor_copy(
                out=hslot_w(0), in_=h0f[:].rearrange("p (m s) -> p m s", m=MT))

            evict_flip = [0]

            def evict_engine():
                evict_flip[0] ^= 1
                return nc.vector if evict_flip[0] else nc.scalar

            # ---------------- chunk prologue ----------------
            stg_cur = [None]

            def emit_chunk(c):
                xt = {e: xtp.tile([P, CHTOK], BF, tag=f"xt{e}", name=f"xt{e}")
                      for e in range(ET)}
                for g in range(CHTOK // P):
                    gi = c * (CHTOK // P) + g
                    xg = gat.tile([P, EMBED], F32, tag="xg", name="xg")
                    nc.gpsimd.indirect_dma_start(
                        out=xg[:], out_offset=None, in_=emb[:],
                        in_offset=bass.IndirectOffsetOnAxis(
                            ap=ids_sb[:, gi:gi + 1], axis=0),
                    )
                    for e in range(ET):
                        tp = ps.tile([P, P], F32, tag="big0", name="tp")
                        nc.tensor.transpose(
                            out=tp[:], in_=xg[:, e * P:(e + 1) * P],
                            identity=ident_f[:])
                        nc.vector.tensor_copy(
                            out=xt[e][:, g * P:(g + 1) * P], in_=tp[:])
                stg = prest.tile([P, SPC * MT * B], BF, tag="prestg",
                                 name="stg")
                stg3 = stg[:].rearrange("p (s m) -> p s m", m=MT * B)
                for mt in range(MT):
                    zp = ps.tile([P, CHTOK], F32, tag=f"big{mt % 2}",
                                 name="zp")
                    for e in range(ET):
                        nc.tensor.matmul(
                            out=zp[:], lhsT=wxh[(e, mt)][:], rhs=xt[e][:],
                            start=(e == 0), stop=False, skip_group_check=True)
                    nc.tensor.matmul(
                        out=zp[:], lhsT=bh[:, mt * P:(mt + 1) * P],
                        rhs=ones_row[:], start=False, stop=True,
                        skip_group_check=True)
                    dst = stg3[:, :, mt * B:(mt + 1) * B]
                    eng = evict_engine()
                    if eng is nc.scalar:
                        nc.scalar.activation(
                            out=dst, in_=zp[:],
                            func=mybir.ActivationFunctionType.Copy)
                    else:
                        nc.vector.tensor_copy(out=dst, in_=zp[:])
                stg_cur[0] = stg

            # ---------------- fc for one token M-tile ----------------
            def emit_fc_mtile(mt):
                t0 = mt * 8
                for vch in range(NCHUNK_V):
                    z = ps.tile([P, PANW], F32, tag=f"big{vch % 2}", name="z")
                    for kt in range(MT):
                        lhsT = hbig[:, kt * MBLK + (t0 + 1) * SLOT:
                                    kt * MBLK + (t0 + 1) * SLOT + P]
                        nc.tensor.matmul(out=z[:], lhsT=lhsT,
                                         rhs=wfc[kt][:, vch * PANW:
                                                     (vch + 1) * PANW],
                                         start=(kt == 0), stop=(kt == MT - 1))
                    ev = evp.tile([P, PANW], F32, tag=f"ev{vch % 4}", name="ev")
                    eng = evict_engine()
                    if eng is nc.scalar:
                        nc.scalar.activation(
                            out=ev[:], in_=z[:],
                            func=mybir.ActivationFunctionType.Copy)
                    else:
                        nc.vector.tensor_copy(out=ev[:], in_=z[:])
                    nc.sync.dma_start(
                        out=out[mt * P:(mt + 1) * P,
                                vch * PANW:(vch + 1) * PANW],
                        in_=ev[:])

            # ---------------- main loop: chunk -> 32 steps -> 4 fc tiles ----
            for c in range(NCH):
                emit_chunk(c)
                for s in range(c * SPC, (c + 1) * SPC):
                    z = ps.tile([P, MT * B], F32, tag="zscan", name="z")
                    nc.tensor.matmul(
                        out=z[:], lhsT=ident_b[:],
                        rhs=stg_cur[0][:, (s % SPC) * 64:(s % SPC) * 64 + 64],
                        start=True, stop=False, skip_group_check=True)
                    for mt in range(MT):
                        for kt in range(MT):
                            nc.tensor.matmul(
                                out=z[:, mt * B:(mt + 1) * B],
                                lhsT=whh[(kt, mt)][:],
                                rhs=hslot_r(s, kt),
                                start=False,
                                stop=(mt == MT - 1 and kt == MT - 1),
                                skip_group_check=True)
                    nc.scalar.activation(
                        out=hslot_w(s + 1), in_=z[:],
                        func=mybir.ActivationFunctionType.Tanh)
                for mt in range(4 * c, 4 * c + 4):
                    emit_fc_mtile(mt)

            stackA.close()
    nc.finalize()
    return nc


def _pack_h(hT):
    # [H, B] -> [128, MT*B] packed (col = m*16+b)
    return np.ascontiguousarray(
        hT.reshape(MT, P, B).transpose(1, 0, 2).reshape(P, MT * B))


def make_in_maps(inputs, h_prev, emb, W_xh_f, W_hh_f, b_h_f,
                 W_xh_b, W_hh_b, b_h_b, W_fc, b_fc):
    inputs = np.asarray(inputs, dtype=np.int32)
    ids = {"f": inputs, "b": inputs[:, ::-1]}
    W_xh = {"f": np.asarray(W_xh_f, np.float32),
            "b": np.asarray(W_xh_b, np.float32)}
    W_hh = {"f": np.asarray(W_hh_f, np.float32),
            "b": np.asarray(W_hh_b, np.float32)}
    b_h = {"f": np.asarray(b_h_f, np.float32),
           "b": np.asarray(b_h_b, np.float32)}
    W_fc = np.asarray(W_fc, np.float32)
    emb = np.ascontiguousarray(np.asarray(emb, dtype=np.float32))
    h0 = _pack_h(np.asarray(h_prev, np.float32).T)

    in_maps = []
    for c in range(NCORES):
        d = "f" if c < 4 else "b"
        j = c % 4
        krows = slice(0, HIDDEN) if d == "f" else slice(HIDDEN, 2 * HIDDEN)
        m = {
            "emb": emb,
            "ids_a": np.ascontiguousarray(ids[d].T.reshape(NG, P).T),
            "whh_a": W_hh[d],
            "wxh_a": W_xh[d],
            "bh_a": np.ascontiguousarray(b_h[d].reshape(1, HIDDEN)),
            "h0": h0,
            "wfc_a": np.ascontiguousarray(
                W_fc[krows, j * VSLICE:(j + 1) * VSLICE]),
        }
        in_maps.append(m)
    return in_maps


def assemble(results, b_fc):
    # core j (fwd) + core j+4 (bwd, time-reversed rows) sum to a vocab slice
    cols = []
    for j in range(4):
        f = results[j]["out"]
        bk = results[j + 4]["out"].reshape(T, B, VSLICE)[::-1].reshape(
            NTOK, VSLICE)
        cols.append(f + bk)
    full = np.concatenate(cols, axis=1)          # [8192, 32000], (t, b) rows
    full = full.reshape(T, B, VOCAB).transpose(1, 0, 2)
    return np.ascontiguousarray(full + np.asarray(b_fc, np.float32))


def kernel(inputs, h_prev, emb, W_xh_f, W_hh_f, b_h_f,
           W_xh_b, W_hh_b, b_h_b, W_fc, b_fc):
    global _CACHED_NC
    if _CACHED_NC is None:
        _CACHED_NC = build()
    in_maps = make_in_maps(inputs, h_prev, emb, W_xh_f, W_hh_f, b_h_f,
                           W_xh_b, W_hh_b, b_h_b, W_fc, b_fc)
    res = run_bass_kernel_spmd(_CACHED_NC, in_maps,
                               core_ids=list(range(NCORES)))
    return assemble(res.results, b_fc)
